# revision 18
# baseline (speedup 1.0000x reference)
"""Trainium2 Bass kernel for nn_MessagePassing_42588895707817.

out = (h @ W.T + b) @ norm_graph,  norm_graph = graph / clip(rowsum(graph), EPS)

Math folding: out = h @ C + 1*d  with  C = W.T @ norm_graph  (128x128),
d = b @ norm_graph. C and d are precomputed on the host in float64 and
shipped as bf16 constants (2M flops - negligible host time).

The problem is HBM-bound (33.5 MB/core fp32 I/O vs ~358 GB/s/core), so all
HBM traffic is bf16: the host downcasts+transposes h to hT [128, 32768]
per core, and the device returns out.T in bf16, which the host transposes
back and upcasts. Traffic halves to 16.8 MB/core (~47us floor). bf16 keeps
rel err ~2e-3, well under the 2e-2 gate.

Device program (per core): C stays the stationary PE operand; each
128-token tile of hT streams through as the moving operand, producing
out.T tiles in PSUM ([g, tok], 4 tiles per bank), which DVE/ACT
(alternating) cast-copy to bf16 SBUF for the store DMA. No PE transposes,
no per-tile weight reloads beyond the self-loading matmul (hidden by
ldw-opt). Loads ride the SP HWDGE ring, stores the ACT ring.

A tiny guard matmul at each chunk start absorbs the input-DMA semaphore
wait so real matmuls only ever wait on the PSUM-bank copy engine (walrus
accepts at most one sync wait on a self-loading Matmult).
"""

import sys

if "/opt/trn_rl_repo" not in sys.path:
    sys.path.insert(0, "/opt/trn_rl_repo")

from contextlib import ExitStack

import ml_dtypes
import numpy as np

B, T, FDIM, HID = 32, 8192, 128, 128
EPS = 1e-10
NCORES = 8
B_LOC = B // NCORES
NTOK = B_LOC * T  # 32768 tokens per core

P = 128  # tokens per PE tile / SBUF partitions
GRP = 4  # tiles per PSUM bank / per copy (engine reads must not cross banks)

BF16 = np.dtype(ml_dtypes.bfloat16)


def build_program(ntok=NTOK, chunk_tiles=32, b_nonzero=False, guard=True,
                  ld_bufs=8, st_bufs=5):
    import concourse.bacc as bacc
    import concourse.tile as tile
    from concourse import mybir

    f32 = mybir.dt.float32
    bf16 = mybir.dt.bfloat16
    ntiles = ntok // P
    nchunks = ntiles // chunk_tiles
    ngroups = chunk_tiles // GRP
    assert ntiles % chunk_tiles == 0 and chunk_tiles % GRP == 0

    nc = bacc.Bacc("TRN2", debug=False, target_bir_lowering=False)

    ht_d = nc.dram_tensor("hT", [FDIM, ntok], bf16, kind="ExternalInput")
    c_d = nc.dram_tensor("C", [FDIM, HID], bf16, kind="ExternalInput")
    if b_nonzero:
        d_d = nc.dram_tensor("d", [1, HID], bf16, kind="ExternalInput")
    out_d = nc.dram_tensor("outT", [HID, ntok], bf16, kind="ExternalOutput")

    # Token tiles are consecutive columns of hT / out.T: chunk c covers
    # columns [c*chunk*128, (c+1)*chunk*128) -> contiguous per-partition DMA
    # runs of chunk*256 bytes on both sides.
    h_v = ht_d[:].rearrange("f (c t p) -> c f t p", t=chunk_tiles, p=P)
    o_v = out_d[:].rearrange("g (c t p) -> c g t p", t=chunk_tiles, p=P)

    with tile.TileContext(nc) as tc, ExitStack() as ctx:
        singles = ctx.enter_context(tc.tile_pool(name="singles", bufs=1))
        ld = ctx.enter_context(tc.tile_pool(name="ld", bufs=min(ld_bufs, ntiles // chunk_tiles)))
        st = ctx.enter_context(tc.tile_pool(name="st", bufs=min(st_bufs, ntiles // chunk_tiles)))
        ps_o = ctx.enter_context(tc.tile_pool(name="ps_o", bufs=6, space="PSUM"))
        if guard:
            ps_g = ctx.enter_context(tc.tile_pool(name="ps_g", bufs=1, space="PSUM"))

        c_raw = singles.tile([P, P], bf16)
        nc.sync.dma_start(out=c_raw, in_=c_d[:])
        # Stage constants through DVE so matmuls never wait on the DMA sem
        # for them after warmup.
        c_s = singles.tile([P, P], bf16)
        nc.vector.tensor_copy(c_s, c_raw)

        if b_nonzero:
            d_raw = singles.tile([1, P], bf16)
            nc.sync.dma_start(out=d_raw, in_=d_d[:])
            d_s = singles.tile([1, P], bf16)
            nc.vector.tensor_copy(d_s, d_raw)
            ones_s = singles.tile([1, P], bf16)
            nc.vector.memset(ones_s, 1.0)

    # out.T tile: psum[g, tok] = sum_f C[f, g] * hT[f, tok]
        for c in range(nchunks):
            in_t = ld.tile([P, chunk_tiles, P], bf16, tag="in_t")
            nc.sync.dma_start(out=in_t, in_=h_v[c])
            out_t = st.tile([P, chunk_tiles, P], bf16)
            if guard:
                # Absorb the input-DMA wait on a throwaway 1x1 matmul so the
                # real matmuls carry only the PSUM-bank (copy engine) wait.
                g_ps = ps_g.tile([1, 1], f32, tag="guard")
                nc.tensor.matmul(g_ps, lhsT=in_t[:, 0, 0:1], rhs=c_s[:, 0:1],
                                 start=True, stop=True)
            for g in range(ngroups):
                o_ps = ps_o.tile([P, GRP, P], f32)
                for j in range(GRP):
                    t = g * GRP + j
                    if b_nonzero:
                        nc.tensor.matmul(o_ps[:, j, :], lhsT=d_s, rhs=ones_s,
                                         start=True, stop=False)
                        nc.tensor.matmul(o_ps[:, j, :], lhsT=c_s,
                                         rhs=in_t[:, t, :], start=False,
                                         stop=True)
                    else:
                        nc.tensor.matmul(o_ps[:, j, :], lhsT=c_s,
                                         rhs=in_t[:, t, :], start=True,
                                         stop=True)
                dst = out_t[:, g * GRP:(g + 1) * GRP, :]
                # Strict alternation keeps both copy engines equally loaded;
                # either engine alone would pace the whole pipeline.
                if g % 2 == 0:
                    nc.scalar.copy(dst, o_ps)
                else:
                    nc.vector.tensor_copy(dst, o_ps)
            # Loads ride the Sync HWDGE ring; stores go SWDGE (gpsimd).
            # Measured best topology: HWDGE-ring stores serialize against
            # the load FIFO or stall the ACT sequencer's copy dispatch;
            # SWDGE stores overlap freely (engine-15 straggler tail is the
            # cheaper price).
            nc.gpsimd.dma_start(out=o_v[c], in_=out_t)

    nc.compile()
    return nc


def make_in_maps(h, graph, W, b, b_nonzero=False):
    g64 = np.asarray(graph, np.float64)
    deg = np.clip(g64.sum(axis=1, keepdims=True), EPS, None)
    ng = np.where(deg > EPS, g64 / deg, 0.0)
    C = (np.asarray(W, np.float64).T @ ng).astype(BF16)  # [F, G]
    hs = np.asarray(h, np.float32).reshape(NCORES, NTOK, FDIM).astype(BF16)
    maps = []
    for i in range(NCORES):
        m = {"hT": np.ascontiguousarray(hs[i].T), "C": C}
        if b_nonzero:
            d = (np.asarray(b, np.float64) @ ng).astype(BF16)
            m["d"] = np.ascontiguousarray(d.reshape(1, HID))
        maps.append(m)
    return maps


_LDW_PATCHED = False


def _enable_ldw_opt(bass_utils):
    """Compile walrus with --enable-ldw-opt=true: lets the PE hide LDWEIGHTS
    behind in-flight matmuls (measured ~3% end-to-end, bit-identical output)."""
    global _LDW_PATCHED
    if _LDW_PATCHED:
        return
    _LDW_PATCHED = True
    orig = bass_utils.run_command

    def patched(argv, **kw):
        argv = [a.replace("--enable-ldw-opt=false", "--enable-ldw-opt=true")
                if isinstance(a, str) else a for a in argv]
        return orig(argv, **kw)

    bass_utils.run_command = patched


def kernel(h, graph, W, b):
    from concourse import bass_utils

    _enable_ldw_opt(bass_utils)
    b_nonzero = bool(np.any(np.asarray(b)))
    nc = build_program(b_nonzero=b_nonzero)
    in_maps = make_in_maps(h, graph, W, b, b_nonzero=b_nonzero)
    res = bass_utils.run_bass_kernel_spmd(nc, in_maps, list(range(NCORES)))
    outs = [
        np.ascontiguousarray(res.results[i]["outT"].T)
        .astype(np.float32)
        .reshape(B_LOC, T, HID)
        for i in range(NCORES)
    ]
    return np.concatenate(outs, axis=0)
